# revision 25
# baseline (speedup 1.0000x reference)
"""Trainium2 Bass kernel for NeuroISNet GNN message passing.

Strategy (8 NeuronCores, one trn2 chip):
  - Batch b -> core pair (2b, 2b+1); each core owns 2048 of 4096 node rows.
  - x[b].T slice lives in SBUF in fp8-e4m3 as 16 pair-tiles [128,2,r];
    the dominant einsum msg = x @ m runs CHUNK-MAJOR as fp8 DoubleRow
    matmuls (K=256 per instruction): each 256-node chunk-pair of m is
    stationary once and feeds 4 accumulating matmuls (one per 512-row
    output block), so partial accumulation starts as each AllGather +
    msg-MLP chunk group lands.
  - The msg MLP runs only on LOCAL nodes; its fp8 output m (scaled by S)
    is what gets AllGathered between the core pair (half the bytes of
    gathering hn, and no duplicated MLP compute).
  - All state is kept transposed (H=128 on partitions, rows on free dim).
  - LayerNorm per 128-row tile via PE transpose + bn_stats; the inv-std
    uses a Quake-style bit-trick rsqrt on the Vector engine so the
    Scalar engine never loads the sqrt table (it only ever runs
    sigmoid/tanh/relu/copy, all in ONE activation table).
  - LSTM elementwise state kept bf16 on the Vector engine.
  - Iteration 1 exploits identical initial rows: msg1 = m0 (x) rowsums.
  - Host does only O(B*H^2) prep: folding weights, init MLP, transposes.
"""

import numpy as np
import ml_dtypes

import concourse.bass as bass
import concourse.mybir as mybir
import concourse.tile as tile
from concourse import bacc
from concourse.bass_utils import run_bass_kernel_spmd

BF = ml_dtypes.bfloat16
F8 = ml_dtypes.float8_e4m3fn
bf16 = mybir.dt.bfloat16
f32 = mybir.dt.float32
i32 = mybir.dt.int32
fp8 = mybir.dt.float8e4

B, N, H, ITERS = 4, 4096, 128, 8
EPS = 1e-5
NCORES = 8
R = N // 2              # rows per core
GROUPS = [[0, 1], [2, 3], [4, 5], [6, 7]]
S = 8.0                 # fp8 scale for m

AF = mybir.ActivationFunctionType
ALU = mybir.AluOpType
DR = mybir.MatmulPerfMode.DoubleRow

MAGIC = 0x5F3759DF      # quake rsqrt magic


def build_module(n_nodes=N, iters=ITERS):
    r = n_nodes // 2            # local rows per core
    kc = n_nodes // 128         # k-chunks (global)
    kc2 = kc // 2               # chunk pairs
    nrb = max(1, r // 512)      # local 512-row blocks
    rbsz = r // nrb             # 512 normally
    ntt = rbsz // 128           # 128-col tiles per rb (4 normally)

    nc = bacc.Bacc("TRN2", target_bir_lowering=False, debug=False,
                   num_devices=NCORES)

    din = lambda name, shape, dt: nc.dram_tensor(name, shape, dt,
                                                 kind="ExternalInput")
    xt_in = din("xt8", [n_nodes, r], fp8)
    h0_in = din("h0", [H, r], bf16)
    rs_in = din("rs", [1, r], bf16)
    m0_in = din("m0", [1, H], bf16)
    b3r_in = din("b3r", [1, H], bf16)
    w1gt_in = din("w1gt", [H, H], bf16)
    w2t_in = din("w2t", [H, H], bf16)
    w3t_in = din("w3t", [H, H], bf16)
    viwt_in = din("vw1gt", [H, H], bf16)
    vw2t_in = din("vw2t", [H, H], bf16)
    vw3t_in = din("vw3t", [H, 1], bf16)
    wiht_in = din("wiht", [H, 4 * H], bf16)
    whht_in = din("whht", [H, 4 * H], bf16)
    b1c_in = din("b1c", [H, 1], f32)
    b2c_in = din("b2c", [H, 1], f32)
    vb1c_in = din("vb1c", [H, 1], f32)
    vb2c_in = din("vb2c", [H, 1], f32)
    bgc_in = din("bgc", [H, 4], f32)
    ident_in = din("ident", [H, H], bf16)

    votes_out = nc.dram_tensor("votes", [1, r], f32, kind="ExternalOutput")

    with tile.TileContext(nc) as tc:
        with tc.tile_pool(name="const", bufs=1) as cp, \
             tc.tile_pool(name="state", bufs=1) as st, \
             tc.tile_pool(name="work", bufs=1) as wk, \
             tc.tile_pool(name="ps", bufs=1, space="PSUM") as ps, \
             tc.tile_pool(name="dram", bufs=1, space="DRAM") as dr:

            # ---- constants ----
            def cload(inp, shape, dt, tag):
                t = cp.tile(shape, dt, tag=tag, name=tag)
                nc.sync.dma_start(t[:], inp[:])
                return t

            w1gt = cload(w1gt_in, [H, H], bf16, "w1gt")
            w2t = cload(w2t_in, [H, H], bf16, "w2t")
            w3t = cload(w3t_in, [H, H], bf16, "w3t")
            vw1gt = cload(viwt_in, [H, H], bf16, "vw1gt")
            vw2t = cload(vw2t_in, [H, H], bf16, "vw2t")
            vw3t = cload(vw3t_in, [H, 1], bf16, "vw3t")
            wiht = cload(wiht_in, [H, 4 * H], bf16, "wiht")
            whht = cload(whht_in, [H, 4 * H], bf16, "whht")
            b1c = cload(b1c_in, [H, 1], f32, "b1c")
            b2c = cload(b2c_in, [H, 1], f32, "b2c")
            vb1c = cload(vb1c_in, [H, 1], f32, "vb1c")
            vb2c = cload(vb2c_in, [H, 1], f32, "vb2c")
            bgc = cload(bgc_in, [H, 4], f32, "bgc")
            ident = cload(ident_in, [H, H], bf16, "ident")
            rs_sb = cload(rs_in, [1, r], bf16, "rs")
            m0_sb = cload(m0_in, [1, H], bf16, "m0")
            b3r = cload(b3r_in, [1, H], bf16, "b3r")

            # ---- state tiles (per rb) ----
            h_rb, c_rb = [], []
            for rb in range(nrb):
                ht = st.tile([H, rbsz], bf16, tag=f"h{rb}", name=f"h{rb}")
                nc.sync.dma_start(ht[:], h0_in[:, rb * rbsz:(rb + 1) * rbsz])
                ct = st.tile([H, rbsz], bf16, tag=f"c{rb}", name=f"c{rb}")
                nc.vector.memset(ct[:], 0.0)
                h_rb.append(ht)
                c_rb.append(ct)
            # m (fp8, xS) for ALL n_nodes as chunk pairs [pair, j, node, h]
            mF8 = st.tile([128, kc2, 2, H], fp8, tag="mF8", name="mF8")
            hnLoc = st.tile([H, r], bf16, tag="hnLoc", name="hnLoc")

            # ---- resident x^T fp8 pair tiles, in consumption order ----
            # (loaded AFTER weights/state so those small DMAs aren't stuck
            #  behind 8MB of x upload; x rides the idle gpsimd DMA queue)
            xtp = [None] * kc2
            order = [p for w in range(2) for p in
                     (list(range(4 * w, 4 * w + 4))
                      + list(range(kc2 // 2 + 4 * w, kc2 // 2 + 4 * w + 4)))]
            for c2 in order:
                t = st.tile([128, 2, r], fp8, tag=f"xtp{c2}", name=f"xtp{c2}")
                for j in range(2):
                    c = 2 * c2 + j
                    nc.gpsimd.dma_start(t[:, j, :],
                                        xt_in[c * 128:(c + 1) * 128, :])
                xtp[c2] = t

            # ---- DRAM bounce buffers for collectives (m, fp8) ----
            cc_in = [dr.tile([128, 2 * ntt * H], fp8, tag=f"cci{w}", bufs=2,
                             name=f"cci{w}") for w in range(2)]
            cc_out = [dr.tile([256, 2 * ntt * H], fp8, tag=f"cco{w}", bufs=2,
                              name=f"cco{w}") for w in range(2)]

            def mlp_pair(r0, r1, it):
                """msg MLP on LOCAL hn blocks r0,r1 -> m8 tiles (interleaved)."""
                m1p, m1s, m2p, m2s, m3p, m8 = {}, {}, {}, {}, {}, {}
                for rb in (r0, r1):
                    src = hnLoc[:, rb * rbsz:(rb + 1) * rbsz]
                    m1p[rb] = ps.tile([H, rbsz], f32, tag="pwork", bufs=2,
                                      name=f"m1p_{it}_{rb}")
                    nc.tensor.matmul(m1p[rb][:], w1gt[:], src,
                                     start=True, stop=True)
                for rb in (r0, r1):
                    m1s[rb] = wk.tile([H, rbsz], bf16, tag="m1s", bufs=2,
                                      name=f"m1s_{it}_{rb}")
                    nc.scalar.activation(m1s[rb][:], m1p[rb][:], AF.Relu,
                                         bias=b1c[:])
                for rb in (r0, r1):
                    m2p[rb] = ps.tile([H, rbsz], f32, tag="pwork", bufs=2,
                                      name=f"m2p_{it}_{rb}")
                    nc.tensor.matmul(m2p[rb][:], w2t[:], m1s[rb][:],
                                     start=True, stop=True)
                for rb in (r0, r1):
                    m2s[rb] = wk.tile([H, rbsz], bf16, tag="m2s", bufs=2,
                                      name=f"m2s_{it}_{rb}")
                    nc.vector.tensor_scalar(m2s[rb][:], m2p[rb][:], b2c[:],
                                            0.0, op0=ALU.add, op1=ALU.max)
                for rb in (r0, r1):
                    m3p[rb] = ps.tile([128, ntt * H], f32, tag="pwork", bufs=2,
                                      name=f"m3p_{it}_{rb}")
                    for t in range(ntt):
                        nc.tensor.matmul(m3p[rb][:, t * H:(t + 1) * H],
                                         m2s[rb][:, t * 128:(t + 1) * 128],
                                         w3t[:], start=True, stop=True)
                for rb in (r0, r1):
                    m8[rb] = wk.tile([128, ntt * H], fp8, tag="m8", bufs=2,
                                     name=f"m8_{it}_{rb}")
                    nc.vector.tensor_scalar(m8[rb][:], m3p[rb][:], S, None,
                                            op0=ALU.mult)
                return m8

            def wave_gather(w, m8, it):
                """AllGather wave w's two m8 blocks; land as mF8 chunk pairs."""
                r0, r1 = 2 * w, 2 * w + 1
                nc.sync.dma_start(cc_in[w][:, 0:ntt * H], m8[r0][:])
                nc.sync.dma_start(cc_in[w][:, ntt * H:2 * ntt * H], m8[r1][:])
                nc.gpsimd.collective_compute(
                    "AllGather", ALU.bypass, replica_groups=GROUPS,
                    ins=[cc_in[w][:].opt()], outs=[cc_out[w][:].opt()])
                # both rank halves land as ready-to-use chunk pairs
                nc.sync.dma_start(mF8[:, 4 * w:4 * w + 4, :, :],
                                  cc_out[w][0:128, :])
                nc.sync.dma_start(
                    mF8[:, kc2 // 2 + 4 * w:kc2 // 2 + 4 * w + 4, :, :],
                    cc_out[w][128:256, :])

            def chunk_block(orb, c2s, msgp, done, it):
                """Accumulate fp8 DoubleRow chunk pairs into ONE msg psum."""
                for c2 in c2s:
                    done[orb] += 1
                    nc.tensor.matmul(
                        msgp[orb][:],
                        mF8[:, c2, :, :],
                        xtp[c2][:, :, orb * rbsz:(orb + 1) * rbsz],
                        start=False, stop=(done[orb] == kc2),
                        perf_mode=DR)

            def lstm_a(rb, mp, it, gact):
                """msgb + gate matmuls + activations for block rb."""
                msgb = wk.tile([H, rbsz], bf16, tag="msgb", bufs=2,
                               name=f"msgb_{it}_{rb}")
                nc.vector.tensor_scalar(msgb[:], mp[:], 1.0 / S, None,
                                        op0=ALU.mult)
                gact[rb] = []
                for g in range(4):
                    gp = ps.tile([H, rbsz], f32, tag="pwork", bufs=2,
                                 name=f"gp_{it}_{rb}_{g}")
                    nc.tensor.matmul(gp[:], wiht[:, g * H:(g + 1) * H],
                                     msgb[:], start=True, stop=False)
                    nc.tensor.matmul(gp[:], whht[:, g * H:(g + 1) * H],
                                     h_rb[rb][:], start=False, stop=True)
                    ga = wk.tile([H, rbsz], bf16, tag=f"ga{g}", bufs=2,
                                 name=f"ga_{it}_{rb}_{g}")
                    nc.scalar.activation(
                        ga[:], gp[:],
                        AF.Tanh if g == 2 else AF.Sigmoid,
                        bias=bgc[:, g:g + 1])
                    gact[rb].append(ga)

            def lstm_c(rb, it, gact):
                """c update on DVE for block rb."""
                si, sf, tg, so = gact[rb]
                t1 = wk.tile([H, rbsz], bf16, tag="t1", bufs=2,
                             name=f"t1_{it}_{rb}")
                nc.vector.tensor_tensor(t1[:], sf[:], c_rb[rb][:], ALU.mult)
                t2 = wk.tile([H, rbsz], bf16, tag="t2", bufs=2,
                             name=f"t2_{it}_{rb}")
                nc.vector.tensor_tensor(t2[:], si[:], tg[:], ALU.mult)
                nc.vector.tensor_tensor(c_rb[rb][:], t1[:], t2[:], ALU.add)

            def lstm_h(rb, it, gact):
                """tanh(c) + h update for block rb."""
                so = gact[rb][3]
                tnc = wk.tile([H, rbsz], bf16, tag="tnc", bufs=2,
                              name=f"tnc_{it}_{rb}")
                nc.scalar.activation(tnc[:], c_rb[rb][:], AF.Tanh)
                nc.vector.tensor_tensor(h_rb[rb][:], so[:], tnc[:], ALU.mult)

            def ln_trp(rb, it, trps):
                """transpose 4 h tiles of block rb into one psum bank."""
                tb = ps.tile([128, 2 * ntt, 128], bf16, tag="ptr", bufs=2,
                             name=f"tb_{it}_{rb}")
                trp4 = tb[:, 0:ntt, :]
                for t in range(ntt):
                    nc.tensor.transpose(
                        trp4[:, t, :], h_rb[rb][:, t * 128:(t + 1) * 128],
                        ident[:])
                trps[rb] = tb

            def ln_stats(rb, it, trps, mvs):
                """bn stats + bit-trick rsqrt (DVE; no scalar sqrt)."""
                trp4 = trps[rb][:, 0:ntt, :]
                st6 = wk.tile([128, ntt, 6], f32, tag="st6", bufs=2,
                              name=f"st6_{it}_{rb}")
                for t in range(ntt):
                    nc.vector.bn_stats(st6[:, t, :], trp4[:, t, :])
                mvb = wk.tile([128, ntt, 2], f32, tag="mvb", bufs=2,
                              name=f"mvb_{it}_{rb}")
                for t in range(ntt):
                    nc.vector.bn_aggr(mvb[:, t, :], st6[:, t, :])
                ve = wk.tile([128, ntt], f32, tag="ve", bufs=2,
                             name=f"ve_{it}_{rb}")
                nc.vector.tensor_scalar(ve[:], mvb[:, :, 1], EPS, None,
                                        op0=ALU.add)
                y0 = wk.tile([128, ntt], f32, tag="y0", bufs=2,
                             name=f"y0_{it}_{rb}")
                ti = wk.tile([128, ntt], i32, tag="ti", bufs=2,
                             name=f"ti_{it}_{rb}")
                nc.vector.tensor_scalar(ti[:], ve[:].bitcast(i32), 1, None,
                                        op0=ALU.logical_shift_right)
                nc.vector.tensor_scalar(y0[:].bitcast(i32), ti[:], MAGIC, -1,
                                        op0=ALU.subtract, op1=ALU.mult)
                aa = wk.tile([128, ntt], f32, tag="aa", bufs=2,
                             name=f"aa_{it}_{rb}")
                nc.vector.tensor_tensor(aa[:], y0[:], y0[:], ALU.mult)
                nc.vector.tensor_tensor(aa[:], ve[:], aa[:], ALU.mult)
                nc.vector.tensor_scalar(aa[:], aa[:], -0.5, 1.5,
                                        op0=ALU.mult, op1=ALU.add)
                sc4 = wk.tile([128, ntt], f32, tag="sc4", bufs=2,
                              name=f"sc4_{it}_{rb}")
                nc.vector.tensor_tensor(sc4[:], y0[:], aa[:], ALU.mult)
                # negated scaled mean so the normalize can run on ScalarE
                nm4 = wk.tile([128, ntt], f32, tag="nm4", bufs=2,
                              name=f"nm4_{it}_{rb}")
                nc.vector.tensor_tensor(nm4[:], mvb[:, :, 0], sc4[:], ALU.mult)
                nc.vector.tensor_scalar(nm4[:], nm4[:], -1.0, None,
                                        op0=ALU.mult)
                mvs[rb] = (mvb, sc4, nm4)

            def ln_hnp(rb, it, trps, mvs):
                """normalize + transpose back + hnLoc copy for block rb."""
                trp4 = trps[rb][:, 0:ntt, :]
                mvb, sc4, nm4 = mvs[rb]
                hnp4 = trps[rb][:, ntt:2 * ntt, :]
                for t in range(ntt):
                    hnr = wk.tile([128, 128], bf16, tag="hnr", bufs=3,
                                  name=f"hnr_{it}_{rb}_{t}")
                    nc.scalar.activation(hnr[:], trp4[:, t, :], AF.Identity,
                                         bias=nm4[:, t:t + 1],
                                         scale=sc4[:, t:t + 1])
                    nc.tensor.transpose(hnp4[:, t, :], hnr[:], ident[:])
                sl = slice(rb * rbsz, (rb + 1) * rbsz)
                nc.vector.tensor_copy(hnLoc[:, sl], hnp4[:])

            # ================= main loop =================
            for it in range(1, iters + 1):
                # rank-1 bias matmuls open each msg psum accumulation group
                msgp = [ps.tile([H, rbsz], f32, tag="pmsg", bufs=4,
                                name=f"msg_{it}_{orb}") for orb in range(nrb)]
                done = [0] * nrb
                if it == 1:
                    for orb in range(nrb):
                        sl = slice(orb * rbsz, (orb + 1) * rbsz)
                        nc.tensor.matmul(msgp[orb][:], m0_sb[:], rs_sb[:, sl],
                                         start=True, stop=True)
                gact, trps, mvs = {}, {}, {}
                w0p = list(range(0, 4)) + list(range(kc2 // 2, kc2 // 2 + 4))
                w1p = [p + 4 for p in w0p]
                if it > 1:
                    for orb in range(nrb):
                        sl = slice(orb * rbsz, (orb + 1) * rbsz)
                        nc.tensor.matmul(msgp[orb][:], b3r[:], rs_sb[:, sl],
                                         start=True, stop=False)
                    # ORB-MAJOR chunks: wave-0 pairs for every block first,
                    # then per block its wave-1 pairs immediately followed by
                    # that block's gate matmuls -- so LSTM/LN/MLP of early
                    # blocks overlap the remaining chunk matmuls and the
                    # AllGathers fire while the PE is still busy
                    for orb in range(nrb):
                        chunk_block(orb, w0p, msgp, done, it)
                    for orb in range(nrb):
                        chunk_block(orb, w1p, msgp, done, it)
                        lstm_a(orb, msgp[orb], it, gact)
                else:
                    for rb in range(nrb):
                        lstm_a(rb, msgp[rb], it, gact)
                for rb in range(nrb):
                    lstm_c(rb, it, gact)
                for rb in range(nrb):
                    lstm_h(rb, it, gact)
                for rb in range(nrb):
                    ln_trp(rb, it, trps)
                    ln_stats(rb, it, trps, mvs)
                    ln_hnp(rb, it, trps, mvs)
                    if rb % 2 == 1 and it < iters:
                        m8 = mlp_pair(rb - 1, rb, it)
                        wave_gather(rb // 2, m8, it)

            # ================= vote =================
            for rb in range(nrb):
                sl = slice(rb * rbsz, (rb + 1) * rbsz)
                v1p = ps.tile([H, rbsz], f32, tag="pmsg", bufs=4,
                              name=f"v1p_{rb}")
                nc.tensor.matmul(v1p[:], vw1gt[:], hnLoc[:, sl],
                                 start=True, stop=True)
                v1s = wk.tile([H, rbsz], bf16, tag="v1s", bufs=1,
                              name=f"v1s_{rb}")
                nc.scalar.activation(v1s[:], v1p[:], AF.Relu, bias=vb1c[:])
                v2p = ps.tile([H, rbsz], f32, tag="pmsg", bufs=4,
                              name=f"v2p_{rb}")
                nc.tensor.matmul(v2p[:], vw2t[:], v1s[:], start=True, stop=True)
                v2s = wk.tile([H, rbsz], bf16, tag="v2s", bufs=1,
                              name=f"v2s_{rb}")
                nc.scalar.activation(v2s[:], v2p[:], AF.Relu, bias=vb2c[:])
                vop = ps.tile([1, rbsz], f32, tag="pwork", bufs=2,
                              name=f"vop_{rb}")
                nc.tensor.matmul(vop[:], vw3t[:], v2s[:], start=True, stop=True)
                vos = wk.tile([1, rbsz], f32, tag="vos", bufs=1,
                              name=f"vos_{rb}")
                nc.scalar.activation(vos[:], vop[:], AF.Copy)
                nc.sync.dma_start(votes_out[:, sl], vos[:])

    nc.compile()
    return nc


_NC_CACHE = {}


def _get_module():
    key = (N, ITERS)
    if key not in _NC_CACHE:
        _NC_CACHE[key] = build_module(N, ITERS)
    return _NC_CACHE[key]


def _host_prep(inputs):
    """Fold weights, run init MLP, build per-core in_maps."""
    g = lambda s: np.asarray(inputs[s], np.float32)
    x = g("x")
    k, n = g("k"), g("n")

    nk = np.stack([k, n], 1)
    a = np.maximum(nk @ g("init_w1").T + g("init_b1"), 0)
    a = np.maximum(a @ g("init_w2").T + g("init_b2"), 0)
    init0 = a @ g("init_w3").T + g("init_b3")          # [B, H]

    ln_g, ln_b = g("ln_g"), g("ln_b")
    mu0 = init0.mean(1, keepdims=True)
    var0 = init0.var(1, keepdims=True)
    embed0 = (init0 - mu0) / np.sqrt(var0 + EPS) * ln_g + ln_b
    t = np.maximum(embed0 @ g("msg_w1").T + g("msg_b1"), 0)
    t = np.maximum(t @ g("msg_w2").T + g("msg_b2"), 0)
    m0eff = t @ g("msg_w3").T + g("msg_b3")            # [B, H]

    com = {
        "w1gt": (g("msg_w1") * ln_g[None, :]).T.astype(BF),
        "w2t": g("msg_w2").T.astype(BF),
        "w3t": g("msg_w3").T.astype(BF),
        "vw1gt": (g("vote_w1") * ln_g[None, :]).T.astype(BF),
        "vw2t": g("vote_w2").T.astype(BF),
        "vw3t": g("vote_w3").T.astype(BF),              # [H, 1]
        "wiht": g("lstm_wih").T.astype(BF),
        "whht": g("lstm_whh").T.astype(BF),
        "b1c": (g("msg_w1") @ ln_b + g("msg_b1")).reshape(H, 1).astype(np.float32),
        "b2c": g("msg_b2").reshape(H, 1).astype(np.float32),
        "vb1c": (g("vote_w1") @ ln_b + g("vote_b1")).reshape(H, 1).astype(np.float32),
        "vb2c": g("vote_b2").reshape(H, 1).astype(np.float32),
        "bgc": (g("lstm_bih") + g("lstm_bhh")).reshape(4, H).T.astype(np.float32).copy(),
        "b3r": (g("msg_b3") * S).reshape(1, H).astype(BF),
        "ident": np.eye(H, dtype=BF),
    }

    in_maps = []
    for core in range(NCORES):
        b = core // 2
        r0 = (core % 2) * R
        xs = x[b][r0:r0 + R, :]                         # [R, N] local rows
        x8 = np.clip(xs.T, -240.0, 240.0).astype(F8)    # [N, R] fp8 (TRN e4)
        m = dict(com)
        m["xt8"] = np.ascontiguousarray(x8)
        m["rs"] = x8.astype(np.float32).sum(0).reshape(1, R).astype(BF)
        m["h0"] = np.ascontiguousarray(
            np.broadcast_to(init0[b][:, None], (H, R))).astype(BF)
        m["m0"] = (m0eff[b] * S).reshape(1, H).astype(BF)
        in_maps.append(m)
    return in_maps


def kernel(**inputs):
    nc = _get_module()
    in_maps = _host_prep(inputs)
    res = run_bass_kernel_spmd(nc, in_maps, core_ids=list(range(NCORES)))
    mask = np.asarray(inputs["mask"], np.float64)
    vb3 = float(np.asarray(inputs["vote_b3"], np.float64).reshape(-1)[0])
    out = np.zeros(B, np.float32)
    for b in range(B):
        votes = np.concatenate([
            res.results[2 * b]["votes"].reshape(-1),
            res.results[2 * b + 1]["votes"].reshape(-1),
        ]).astype(np.float64) + vb3
        s = float((votes * mask[b]).sum())
        out[b] = 1.0 / (1.0 + np.exp(-s))
    return out


# revision 27
# speedup vs baseline: 1.0024x; 1.0024x over previous
"""Trainium2 Bass kernel for NeuroISNet GNN message passing.

Strategy (8 NeuronCores, one trn2 chip):
  - Batch b -> core pair (2b, 2b+1); each core owns 2048 of 4096 node rows.
  - x[b].T slice lives in SBUF in fp8-e4m3 as 16 pair-tiles [128,2,r];
    the dominant einsum msg = x @ m runs CHUNK-MAJOR as fp8 DoubleRow
    matmuls (K=256 per instruction): each 256-node chunk-pair of m is
    stationary once and feeds 4 accumulating matmuls (one per 512-row
    output block), so partial accumulation starts as each AllGather +
    msg-MLP chunk group lands.
  - The msg MLP runs only on LOCAL nodes; its fp8 output m (scaled by S)
    is what gets AllGathered between the core pair (half the bytes of
    gathering hn, and no duplicated MLP compute).
  - All state is kept transposed (H=128 on partitions, rows on free dim).
  - LayerNorm per 128-row tile via PE transpose + bn_stats; the inv-std
    uses a Quake-style bit-trick rsqrt on the Vector engine so the
    Scalar engine never loads the sqrt table (it only ever runs
    sigmoid/tanh/relu/copy, all in ONE activation table).
  - LSTM elementwise state kept bf16 on the Vector engine.
  - Iteration 1 exploits identical initial rows: msg1 = m0 (x) rowsums.
  - Host does only O(B*H^2) prep: folding weights, init MLP, transposes.
"""

import numpy as np
import ml_dtypes

import concourse.bass as bass
import concourse.mybir as mybir
import concourse.tile as tile
from concourse import bacc
from concourse.bass_utils import run_bass_kernel_spmd

BF = ml_dtypes.bfloat16
F8 = ml_dtypes.float8_e4m3fn
bf16 = mybir.dt.bfloat16
f32 = mybir.dt.float32
i32 = mybir.dt.int32
fp8 = mybir.dt.float8e4

B, N, H, ITERS = 4, 4096, 128, 8
EPS = 1e-5
NCORES = 8
R = N // 2              # rows per core
GROUPS = [[0, 1], [2, 3], [4, 5], [6, 7]]
S = 8.0                 # fp8 scale for m

AF = mybir.ActivationFunctionType
ALU = mybir.AluOpType
DR = mybir.MatmulPerfMode.DoubleRow

MAGIC = 0x5F3759DF      # quake rsqrt magic


def build_module(n_nodes=N, iters=ITERS):
    r = n_nodes // 2            # local rows per core
    kc = n_nodes // 128         # k-chunks (global)
    kc2 = kc // 2               # chunk pairs
    nrb = max(1, r // 512)      # local 512-row blocks
    rbsz = r // nrb             # 512 normally
    ntt = rbsz // 128           # 128-col tiles per rb (4 normally)

    nc = bacc.Bacc("TRN2", target_bir_lowering=False, debug=False,
                   num_devices=NCORES)

    din = lambda name, shape, dt: nc.dram_tensor(name, shape, dt,
                                                 kind="ExternalInput")
    xt_in = din("xt8", [n_nodes, r], fp8)
    h0_in = din("h0", [H, r], bf16)
    rs_in = din("rs", [1, r], bf16)
    m0_in = din("m0", [1, H], bf16)
    b3r_in = din("b3r", [1, H], bf16)
    w1gt_in = din("w1gt", [H, H], bf16)
    w2t_in = din("w2t", [H, H], bf16)
    w3t_in = din("w3t", [H, H], bf16)
    viwt_in = din("vw1gt", [H, H], bf16)
    vw2t_in = din("vw2t", [H, H], bf16)
    vw3t_in = din("vw3t", [H, 1], bf16)
    wiht_in = din("wiht", [H, 4 * H], bf16)
    whht_in = din("whht", [H, 4 * H], bf16)
    b1c_in = din("b1c", [H, 1], f32)
    b2c_in = din("b2c", [H, 1], f32)
    vb1c_in = din("vb1c", [H, 1], f32)
    vb2c_in = din("vb2c", [H, 1], f32)
    bgc_in = din("bgc", [H, 4], f32)
    ident_in = din("ident", [H, H], bf16)

    votes_out = nc.dram_tensor("votes", [1, r], f32, kind="ExternalOutput")

    with tile.TileContext(nc) as tc:
        with tc.tile_pool(name="const", bufs=1) as cp, \
             tc.tile_pool(name="state", bufs=1) as st, \
             tc.tile_pool(name="work", bufs=1) as wk, \
             tc.tile_pool(name="ps", bufs=1, space="PSUM") as ps, \
             tc.tile_pool(name="dram", bufs=1, space="DRAM") as dr:

            # ---- constants ----
            def cload(inp, shape, dt, tag):
                t = cp.tile(shape, dt, tag=tag, name=tag)
                nc.sync.dma_start(t[:], inp[:])
                return t

            w1gt = cload(w1gt_in, [H, H], bf16, "w1gt")
            w2t = cload(w2t_in, [H, H], bf16, "w2t")
            w3t = cload(w3t_in, [H, H], bf16, "w3t")
            vw1gt = cload(viwt_in, [H, H], bf16, "vw1gt")
            vw2t = cload(vw2t_in, [H, H], bf16, "vw2t")
            vw3t = cload(vw3t_in, [H, 1], bf16, "vw3t")
            wiht = cload(wiht_in, [H, 4 * H], bf16, "wiht")
            whht = cload(whht_in, [H, 4 * H], bf16, "whht")
            b1c = cload(b1c_in, [H, 1], f32, "b1c")
            b2c = cload(b2c_in, [H, 1], f32, "b2c")
            vb1c = cload(vb1c_in, [H, 1], f32, "vb1c")
            vb2c = cload(vb2c_in, [H, 1], f32, "vb2c")
            bgc = cload(bgc_in, [H, 4], f32, "bgc")
            ident = cload(ident_in, [H, H], bf16, "ident")
            rs_sb = cload(rs_in, [1, r], bf16, "rs")
            m0_sb = cload(m0_in, [1, H], bf16, "m0")
            b3r = cload(b3r_in, [1, H], bf16, "b3r")

            # ---- state tiles (per rb) ----
            h_rb, c_rb = [], []
            for rb in range(nrb):
                ht = st.tile([H, rbsz], bf16, tag=f"h{rb}", name=f"h{rb}")
                nc.sync.dma_start(ht[:], h0_in[:, rb * rbsz:(rb + 1) * rbsz])
                ct = st.tile([H, rbsz], bf16, tag=f"c{rb}", name=f"c{rb}")
                nc.vector.memset(ct[:], 0.0)
                h_rb.append(ht)
                c_rb.append(ct)
            # m (fp8, xS) for ALL n_nodes as chunk pairs [pair, j, node, h]
            mF8 = st.tile([128, kc2, 2, H], fp8, tag="mF8", name="mF8")
            hnLoc = st.tile([H, r], bf16, tag="hnLoc", name="hnLoc")

            # ---- resident x^T fp8 pair tiles, in consumption order ----
            # (loaded AFTER weights/state so those small DMAs aren't stuck
            #  behind 8MB of x upload; x rides the idle gpsimd DMA queue)
            xtp = [None] * kc2
            order = [p for w in range(2) for p in
                     (list(range(4 * w, 4 * w + 4))
                      + list(range(kc2 // 2 + 4 * w, kc2 // 2 + 4 * w + 4)))]
            for c2 in order:
                t = st.tile([128, 2, r], fp8, tag=f"xtp{c2}", name=f"xtp{c2}")
                for j in range(2):
                    c = 2 * c2 + j
                    nc.gpsimd.dma_start(t[:, j, :],
                                        xt_in[c * 128:(c + 1) * 128, :])
                xtp[c2] = t

            # ---- DRAM bounce buffers for collectives (m, fp8) ----
            cc_in = [dr.tile([128, 2 * ntt * H], fp8, tag=f"cci{w}", bufs=2,
                             name=f"cci{w}") for w in range(2)]
            cc_out = [dr.tile([256, 2 * ntt * H], fp8, tag=f"cco{w}", bufs=2,
                              name=f"cco{w}") for w in range(2)]

            def mlp_pair(r0, r1, it):
                """msg MLP on LOCAL hn blocks r0,r1 -> m8 tiles (interleaved)."""
                m1p, m1s, m2p, m2s, m3p, m8 = {}, {}, {}, {}, {}, {}
                for rb in (r0, r1):
                    src = hnLoc[:, rb * rbsz:(rb + 1) * rbsz]
                    m1p[rb] = ps.tile([H, rbsz], f32, tag="pwork", bufs=2,
                                      name=f"m1p_{it}_{rb}")
                    nc.tensor.matmul(m1p[rb][:], w1gt[:], src,
                                     start=True, stop=True)
                for rb in (r0, r1):
                    m1s[rb] = wk.tile([H, rbsz], bf16, tag="m1s", bufs=2,
                                      name=f"m1s_{it}_{rb}")
                    nc.scalar.activation(m1s[rb][:], m1p[rb][:], AF.Relu,
                                         bias=b1c[:])
                for rb in (r0, r1):
                    m2p[rb] = ps.tile([H, rbsz], f32, tag="pwork", bufs=2,
                                      name=f"m2p_{it}_{rb}")
                    nc.tensor.matmul(m2p[rb][:], w2t[:], m1s[rb][:],
                                     start=True, stop=True)
                for rb in (r0, r1):
                    m2s[rb] = wk.tile([H, rbsz], bf16, tag="m2s", bufs=2,
                                      name=f"m2s_{it}_{rb}")
                    nc.vector.tensor_scalar(m2s[rb][:], m2p[rb][:], b2c[:],
                                            0.0, op0=ALU.add, op1=ALU.max)
                for rb in (r0, r1):
                    m3p[rb] = ps.tile([128, ntt * H], f32, tag="pwork", bufs=2,
                                      name=f"m3p_{it}_{rb}")
                    for t in range(ntt):
                        nc.tensor.matmul(m3p[rb][:, t * H:(t + 1) * H],
                                         m2s[rb][:, t * 128:(t + 1) * 128],
                                         w3t[:], start=True, stop=True)
                for rb in (r0, r1):
                    m8[rb] = wk.tile([128, ntt * H], fp8, tag="m8", bufs=2,
                                     name=f"m8_{it}_{rb}")
                    nc.vector.tensor_scalar(m8[rb][:], m3p[rb][:], S, None,
                                            op0=ALU.mult)
                return m8

            def wave_gather(w, m8, it):
                """AllGather wave w's two m8 blocks; land as mF8 chunk pairs."""
                r0, r1 = 2 * w, 2 * w + 1
                nc.sync.dma_start(cc_in[w][:, 0:ntt * H], m8[r0][:])
                nc.sync.dma_start(cc_in[w][:, ntt * H:2 * ntt * H], m8[r1][:])
                nc.gpsimd.collective_compute(
                    "AllGather", ALU.bypass, replica_groups=GROUPS,
                    ins=[cc_in[w][:].opt()], outs=[cc_out[w][:].opt()])
                # both rank halves land as ready-to-use chunk pairs
                nc.sync.dma_start(mF8[:, 4 * w:4 * w + 4, :, :],
                                  cc_out[w][0:128, :])
                nc.sync.dma_start(
                    mF8[:, kc2 // 2 + 4 * w:kc2 // 2 + 4 * w + 4, :, :],
                    cc_out[w][128:256, :])

            def chunk_block(orb, c2s, msgp, done, it):
                """Accumulate fp8 DoubleRow chunk pairs into ONE msg psum."""
                for c2 in c2s:
                    done[orb] += 1
                    nc.tensor.matmul(
                        msgp[orb][:],
                        mF8[:, c2, :, :],
                        xtp[c2][:, :, orb * rbsz:(orb + 1) * rbsz],
                        start=False, stop=(done[orb] == kc2),
                        perf_mode=DR)

            def lstm_a(rb, mp, it, gact):
                """msgb + gate matmuls + activations for block rb."""
                msgb = wk.tile([H, rbsz], bf16, tag="msgb", bufs=2,
                               name=f"msgb_{it}_{rb}")
                nc.vector.tensor_scalar(msgb[:], mp[:], 1.0 / S, None,
                                        op0=ALU.mult)
                gact[rb] = []
                for g in range(4):
                    gp = ps.tile([H, rbsz], f32, tag="pwork", bufs=2,
                                 name=f"gp_{it}_{rb}_{g}")
                    nc.tensor.matmul(gp[:], wiht[:, g * H:(g + 1) * H],
                                     msgb[:], start=True, stop=False)
                    nc.tensor.matmul(gp[:], whht[:, g * H:(g + 1) * H],
                                     h_rb[rb][:], start=False, stop=True)
                    ga = wk.tile([H, rbsz], bf16, tag=f"ga{g}", bufs=2,
                                 name=f"ga_{it}_{rb}_{g}")
                    nc.scalar.activation(
                        ga[:], gp[:],
                        AF.Tanh if g == 2 else AF.Sigmoid,
                        bias=bgc[:, g:g + 1])
                    gact[rb].append(ga)

            def lstm_c(rb, it, gact):
                """c update on DVE for block rb."""
                si, sf, tg, so = gact[rb]
                t1 = wk.tile([H, rbsz], bf16, tag="t1", bufs=2,
                             name=f"t1_{it}_{rb}")
                nc.vector.tensor_tensor(t1[:], sf[:], c_rb[rb][:], ALU.mult)
                t2 = wk.tile([H, rbsz], bf16, tag="t2", bufs=2,
                             name=f"t2_{it}_{rb}")
                nc.vector.tensor_tensor(t2[:], si[:], tg[:], ALU.mult)
                nc.vector.tensor_tensor(c_rb[rb][:], t1[:], t2[:], ALU.add)

            def lstm_h(rb, it, gact):
                """tanh(c) + h update for block rb."""
                so = gact[rb][3]
                tnc = wk.tile([H, rbsz], bf16, tag="tnc", bufs=2,
                              name=f"tnc_{it}_{rb}")
                nc.scalar.activation(tnc[:], c_rb[rb][:], AF.Tanh)
                nc.vector.tensor_tensor(h_rb[rb][:], so[:], tnc[:], ALU.mult)

            def ln_trp(rb, it, trps):
                """transpose 4 h tiles of block rb into one psum bank."""
                tb = ps.tile([128, 2 * ntt, 128], bf16, tag="ptr", bufs=2,
                             name=f"tb_{it}_{rb}")
                trp4 = tb[:, 0:ntt, :]
                for t in range(ntt):
                    nc.tensor.transpose(
                        trp4[:, t, :], h_rb[rb][:, t * 128:(t + 1) * 128],
                        ident[:])
                trps[rb] = tb

            def ln_stats(rb, it, trps, mvs):
                """bn stats + bit-trick rsqrt (DVE; no scalar sqrt)."""
                trp4 = trps[rb][:, 0:ntt, :]
                st6 = wk.tile([128, ntt, 6], f32, tag="st6", bufs=2,
                              name=f"st6_{it}_{rb}")
                for t in range(ntt):
                    nc.vector.bn_stats(st6[:, t, :], trp4[:, t, :])
                mvb = wk.tile([128, ntt, 2], f32, tag="mvb", bufs=2,
                              name=f"mvb_{it}_{rb}")
                for t in range(ntt):
                    nc.vector.bn_aggr(mvb[:, t, :], st6[:, t, :])
                ve = wk.tile([128, ntt], f32, tag="ve", bufs=2,
                             name=f"ve_{it}_{rb}")
                nc.vector.tensor_scalar(ve[:], mvb[:, :, 1], EPS, None,
                                        op0=ALU.add)
                y0 = wk.tile([128, ntt], f32, tag="y0", bufs=2,
                             name=f"y0_{it}_{rb}")
                ti = wk.tile([128, ntt], i32, tag="ti", bufs=2,
                             name=f"ti_{it}_{rb}")
                nc.vector.tensor_scalar(ti[:], ve[:].bitcast(i32), 1, None,
                                        op0=ALU.logical_shift_right)
                nc.vector.tensor_scalar(y0[:].bitcast(i32), ti[:], MAGIC, -1,
                                        op0=ALU.subtract, op1=ALU.mult)
                aa = wk.tile([128, ntt], f32, tag="aa", bufs=2,
                             name=f"aa_{it}_{rb}")
                nc.vector.tensor_tensor(aa[:], y0[:], y0[:], ALU.mult)
                nc.vector.tensor_tensor(aa[:], ve[:], aa[:], ALU.mult)
                nc.vector.tensor_scalar(aa[:], aa[:], -0.5, 1.5,
                                        op0=ALU.mult, op1=ALU.add)
                sc4 = wk.tile([128, ntt], f32, tag="sc4", bufs=2,
                              name=f"sc4_{it}_{rb}")
                nc.vector.tensor_tensor(sc4[:], y0[:], aa[:], ALU.mult)
                mvs[rb] = (mvb, sc4)

            def ln_hnp(rb, it, trps, mvs):
                """normalize + transpose back + hnLoc copy for block rb."""
                trp4 = trps[rb][:, 0:ntt, :]
                mvb, sc4 = mvs[rb]
                hnp4 = trps[rb][:, ntt:2 * ntt, :]
                for t in range(ntt):
                    hnr = wk.tile([128, 128], bf16, tag="hnr", bufs=3,
                                  name=f"hnr_{it}_{rb}_{t}")
                    nc.vector.tensor_scalar(hnr[:], trp4[:, t, :],
                                            mvb[:, t, 0:1], sc4[:, t:t + 1],
                                            op0=ALU.subtract, op1=ALU.mult)
                    nc.tensor.transpose(hnp4[:, t, :], hnr[:], ident[:])
                sl = slice(rb * rbsz, (rb + 1) * rbsz)
                nc.vector.tensor_copy(hnLoc[:, sl], hnp4[:])

            # ================= main loop =================
            for it in range(1, iters + 1):
                # rank-1 bias matmuls open each msg psum accumulation group
                msgp = [ps.tile([H, rbsz], f32, tag="pmsg", bufs=4,
                                name=f"msg_{it}_{orb}") for orb in range(nrb)]
                done = [0] * nrb
                if it == 1:
                    for orb in range(nrb):
                        sl = slice(orb * rbsz, (orb + 1) * rbsz)
                        nc.tensor.matmul(msgp[orb][:], m0_sb[:], rs_sb[:, sl],
                                         start=True, stop=True)
                gact, trps, mvs = {}, {}, {}
                w0p = list(range(0, 4)) + list(range(kc2 // 2, kc2 // 2 + 4))
                w1p = [p + 4 for p in w0p]
                if it > 1:
                    for orb in range(nrb):
                        sl = slice(orb * rbsz, (orb + 1) * rbsz)
                        nc.tensor.matmul(msgp[orb][:], b3r[:], rs_sb[:, sl],
                                         start=True, stop=False)
                    # ORB-MAJOR chunks: wave-0 pairs for every block first,
                    # then per block its wave-1 pairs immediately followed by
                    # that block's gate matmuls -- so LSTM/LN/MLP of early
                    # blocks overlap the remaining chunk matmuls and the
                    # AllGathers fire while the PE is still busy
                    for orb in range(nrb):
                        chunk_block(orb, w0p, msgp, done, it)
                    # per block: chunks -> gates -> cell update, with the
                    # h-update one block behind so the scalar FIFO never
                    # stalls on the DVE cell chain; h(rb) lands well before
                    # the PE drains the chunk phase (kills the post-chunk gap)
                    for orb in range(nrb):
                        chunk_block(orb, w1p, msgp, done, it)
                        lstm_a(orb, msgp[orb], it, gact)
                        lstm_c(orb, it, gact)
                        if orb >= 1:
                            lstm_h(orb - 1, it, gact)
                    lstm_h(nrb - 1, it, gact)
                else:
                    for rb in range(nrb):
                        lstm_a(rb, msgp[rb], it, gact)
                    for rb in range(nrb):
                        lstm_c(rb, it, gact)
                    for rb in range(nrb):
                        lstm_h(rb, it, gact)
                for rb in range(nrb):
                    ln_trp(rb, it, trps)
                    ln_stats(rb, it, trps, mvs)
                    ln_hnp(rb, it, trps, mvs)
                    if rb % 2 == 1 and it < iters:
                        m8 = mlp_pair(rb - 1, rb, it)
                        wave_gather(rb // 2, m8, it)

            # ================= vote =================
            for rb in range(nrb):
                sl = slice(rb * rbsz, (rb + 1) * rbsz)
                v1p = ps.tile([H, rbsz], f32, tag="pmsg", bufs=4,
                              name=f"v1p_{rb}")
                nc.tensor.matmul(v1p[:], vw1gt[:], hnLoc[:, sl],
                                 start=True, stop=True)
                v1s = wk.tile([H, rbsz], bf16, tag="v1s", bufs=1,
                              name=f"v1s_{rb}")
                nc.scalar.activation(v1s[:], v1p[:], AF.Relu, bias=vb1c[:])
                v2p = ps.tile([H, rbsz], f32, tag="pmsg", bufs=4,
                              name=f"v2p_{rb}")
                nc.tensor.matmul(v2p[:], vw2t[:], v1s[:], start=True, stop=True)
                v2s = wk.tile([H, rbsz], bf16, tag="v2s", bufs=1,
                              name=f"v2s_{rb}")
                nc.scalar.activation(v2s[:], v2p[:], AF.Relu, bias=vb2c[:])
                vop = ps.tile([1, rbsz], f32, tag="pwork", bufs=2,
                              name=f"vop_{rb}")
                nc.tensor.matmul(vop[:], vw3t[:], v2s[:], start=True, stop=True)
                vos = wk.tile([1, rbsz], f32, tag="vos", bufs=1,
                              name=f"vos_{rb}")
                nc.scalar.activation(vos[:], vop[:], AF.Copy)
                nc.sync.dma_start(votes_out[:, sl], vos[:])

    nc.compile()
    return nc


_NC_CACHE = {}


def _get_module():
    key = (N, ITERS)
    if key not in _NC_CACHE:
        _NC_CACHE[key] = build_module(N, ITERS)
    return _NC_CACHE[key]


def _host_prep(inputs):
    """Fold weights, run init MLP, build per-core in_maps."""
    g = lambda s: np.asarray(inputs[s], np.float32)
    x = g("x")
    k, n = g("k"), g("n")

    nk = np.stack([k, n], 1)
    a = np.maximum(nk @ g("init_w1").T + g("init_b1"), 0)
    a = np.maximum(a @ g("init_w2").T + g("init_b2"), 0)
    init0 = a @ g("init_w3").T + g("init_b3")          # [B, H]

    ln_g, ln_b = g("ln_g"), g("ln_b")
    mu0 = init0.mean(1, keepdims=True)
    var0 = init0.var(1, keepdims=True)
    embed0 = (init0 - mu0) / np.sqrt(var0 + EPS) * ln_g + ln_b
    t = np.maximum(embed0 @ g("msg_w1").T + g("msg_b1"), 0)
    t = np.maximum(t @ g("msg_w2").T + g("msg_b2"), 0)
    m0eff = t @ g("msg_w3").T + g("msg_b3")            # [B, H]

    com = {
        "w1gt": (g("msg_w1") * ln_g[None, :]).T.astype(BF),
        "w2t": g("msg_w2").T.astype(BF),
        "w3t": g("msg_w3").T.astype(BF),
        "vw1gt": (g("vote_w1") * ln_g[None, :]).T.astype(BF),
        "vw2t": g("vote_w2").T.astype(BF),
        "vw3t": g("vote_w3").T.astype(BF),              # [H, 1]
        "wiht": g("lstm_wih").T.astype(BF),
        "whht": g("lstm_whh").T.astype(BF),
        "b1c": (g("msg_w1") @ ln_b + g("msg_b1")).reshape(H, 1).astype(np.float32),
        "b2c": g("msg_b2").reshape(H, 1).astype(np.float32),
        "vb1c": (g("vote_w1") @ ln_b + g("vote_b1")).reshape(H, 1).astype(np.float32),
        "vb2c": g("vote_b2").reshape(H, 1).astype(np.float32),
        "bgc": (g("lstm_bih") + g("lstm_bhh")).reshape(4, H).T.astype(np.float32).copy(),
        "b3r": (g("msg_b3") * S).reshape(1, H).astype(BF),
        "ident": np.eye(H, dtype=BF),
    }

    in_maps = []
    for core in range(NCORES):
        b = core // 2
        r0 = (core % 2) * R
        xs = x[b][r0:r0 + R, :]                         # [R, N] local rows
        x8 = np.clip(xs.T, -240.0, 240.0).astype(F8)    # [N, R] fp8 (TRN e4)
        m = dict(com)
        m["xt8"] = np.ascontiguousarray(x8)
        m["rs"] = x8.astype(np.float32).sum(0).reshape(1, R).astype(BF)
        m["h0"] = np.ascontiguousarray(
            np.broadcast_to(init0[b][:, None], (H, R))).astype(BF)
        m["m0"] = (m0eff[b] * S).reshape(1, H).astype(BF)
        in_maps.append(m)
    return in_maps


def kernel(**inputs):
    nc = _get_module()
    in_maps = _host_prep(inputs)
    res = run_bass_kernel_spmd(nc, in_maps, core_ids=list(range(NCORES)))
    mask = np.asarray(inputs["mask"], np.float64)
    vb3 = float(np.asarray(inputs["vote_b3"], np.float64).reshape(-1)[0])
    out = np.zeros(B, np.float32)
    for b in range(B):
        votes = np.concatenate([
            res.results[2 * b]["votes"].reshape(-1),
            res.results[2 * b + 1]["votes"].reshape(-1),
        ]).astype(np.float64) + vb3
        s = float((votes * mask[b]).sum())
        out[b] = 1.0 / (1.0 + np.exp(-s))
    return out


# revision 31
# speedup vs baseline: 1.0157x; 1.0133x over previous
"""Trainium2 Bass kernel for NeuroISNet GNN message passing.

Strategy (8 NeuronCores, one trn2 chip):
  - Batch b -> core pair (2b, 2b+1); each core owns 2048 of 4096 node rows.
  - x[b].T slice lives in SBUF in fp8-e4m3 as 16 pair-tiles [128,2,r];
    the dominant einsum msg = x @ m runs CHUNK-MAJOR as fp8 DoubleRow
    matmuls (K=256 per instruction): each 256-node chunk-pair of m is
    stationary once and feeds 4 accumulating matmuls (one per 512-row
    output block), so partial accumulation starts as each AllGather +
    msg-MLP chunk group lands.
  - The msg MLP runs only on LOCAL nodes; its fp8 output m (scaled by S)
    is what gets AllGathered between the core pair (half the bytes of
    gathering hn, and no duplicated MLP compute).
  - All state is kept transposed (H=128 on partitions, rows on free dim).
  - LayerNorm per 128-row tile via PE transpose + bn_stats; the inv-std
    uses a Quake-style bit-trick rsqrt on the Vector engine so the
    Scalar engine never loads the sqrt table (it only ever runs
    sigmoid/tanh/relu/copy, all in ONE activation table).
  - LSTM elementwise state kept bf16 on the Vector engine.
  - Iteration 1 exploits identical initial rows: msg1 = m0 (x) rowsums.
  - Host does only O(B*H^2) prep: folding weights, init MLP, transposes.
"""

import numpy as np
import ml_dtypes

import concourse.bass as bass
import concourse.mybir as mybir
import concourse.tile as tile
from concourse import bacc
from concourse.bass_utils import run_bass_kernel_spmd

BF = ml_dtypes.bfloat16
F8 = ml_dtypes.float8_e4m3fn
bf16 = mybir.dt.bfloat16
f32 = mybir.dt.float32
i32 = mybir.dt.int32
fp8 = mybir.dt.float8e4

B, N, H, ITERS = 4, 4096, 128, 8
EPS = 1e-5
NCORES = 8
R = N // 2              # rows per core
GROUPS = [[0, 1], [2, 3], [4, 5], [6, 7]]
S = 8.0                 # fp8 scale for m

AF = mybir.ActivationFunctionType
ALU = mybir.AluOpType
DR = mybir.MatmulPerfMode.DoubleRow

MAGIC = 0x5F3759DF      # quake rsqrt magic


def build_module(n_nodes=N, iters=ITERS):
    r = n_nodes // 2            # local rows per core
    kc = n_nodes // 128         # k-chunks (global)
    kc2 = kc // 2               # chunk pairs
    nrb = max(1, r // 512)      # local 512-row blocks
    rbsz = r // nrb             # 512 normally
    ntt = rbsz // 128           # 128-col tiles per rb (4 normally)

    nc = bacc.Bacc("TRN2", target_bir_lowering=False, debug=False,
                   num_devices=NCORES)

    din = lambda name, shape, dt: nc.dram_tensor(name, shape, dt,
                                                 kind="ExternalInput")
    xt_in = din("xt8", [n_nodes, r], fp8)
    h0_in = din("h0", [H, r], bf16)
    rs_in = din("rs", [1, r], bf16)
    m0_in = din("m0", [1, H], bf16)
    b3r_in = din("b3r", [1, H], bf16)
    w1gt_in = din("w1gt", [H, H], bf16)
    w2t_in = din("w2t", [H, H], bf16)
    w3t_in = din("w3t", [H, H], bf16)
    viwt_in = din("vw1gt", [H, H], bf16)
    vw2t_in = din("vw2t", [H, H], bf16)
    vw3t_in = din("vw3t", [H, 1], bf16)
    wiht_in = din("wiht", [H, 4 * H], bf16)
    whht_in = din("whht", [H, 4 * H], bf16)
    b1c_in = din("b1c", [H, 1], f32)
    b2c_in = din("b2c", [H, 1], f32)
    vb1c_in = din("vb1c", [H, 1], f32)
    vb2c_in = din("vb2c", [H, 1], f32)
    bgc_in = din("bgc", [H, 4], f32)
    ident_in = din("ident", [H, H], bf16)

    votes_out = nc.dram_tensor("votes", [1, r], f32, kind="ExternalOutput")

    with tile.TileContext(nc) as tc:
        with tc.tile_pool(name="const", bufs=1) as cp, \
             tc.tile_pool(name="state", bufs=1) as st, \
             tc.tile_pool(name="work", bufs=1) as wk, \
             tc.tile_pool(name="ps", bufs=1, space="PSUM") as ps, \
             tc.tile_pool(name="dram", bufs=1, space="DRAM") as dr:

            # ---- constants ----
            def cload(inp, shape, dt, tag):
                t = cp.tile(shape, dt, tag=tag, name=tag)
                nc.sync.dma_start(t[:], inp[:])
                return t

            w1gt = cload(w1gt_in, [H, H], bf16, "w1gt")
            w2t = cload(w2t_in, [H, H], bf16, "w2t")
            w3t = cload(w3t_in, [H, H], bf16, "w3t")
            vw1gt = cload(viwt_in, [H, H], bf16, "vw1gt")
            vw2t = cload(vw2t_in, [H, H], bf16, "vw2t")
            vw3t = cload(vw3t_in, [H, 1], bf16, "vw3t")
            wiht = cload(wiht_in, [H, 4 * H], bf16, "wiht")
            whht = cload(whht_in, [H, 4 * H], bf16, "whht")
            b1c = cload(b1c_in, [H, 1], f32, "b1c")
            b2c = cload(b2c_in, [H, 1], f32, "b2c")
            vb1c = cload(vb1c_in, [H, 1], f32, "vb1c")
            vb2c = cload(vb2c_in, [H, 1], f32, "vb2c")
            bgc = cload(bgc_in, [H, 4], f32, "bgc")
            ident = cload(ident_in, [H, H], bf16, "ident")
            rs_sb = cload(rs_in, [1, r], bf16, "rs")
            m0_sb = cload(m0_in, [1, H], bf16, "m0")
            b3r = cload(b3r_in, [1, H], bf16, "b3r")

            # ---- state tiles (per rb) ----
            h_rb, c_rb = [], []
            for rb in range(nrb):
                ht = st.tile([H, rbsz], bf16, tag=f"h{rb}", name=f"h{rb}")
                nc.sync.dma_start(ht[:], h0_in[:, rb * rbsz:(rb + 1) * rbsz])
                ct = st.tile([H, rbsz], bf16, tag=f"c{rb}", name=f"c{rb}")
                nc.vector.memset(ct[:], 0.0)
                h_rb.append(ht)
                c_rb.append(ct)
            # m (fp8, xS) for ALL n_nodes as chunk pairs [pair, j, node, h],
            # SPLIT per gather wave so wave-0 chunk reads carry no false
            # dependency on the wave-1 gather DMA landing in the same tile
            mF8w = [st.tile([128, kc2 // 2, 2, H], fp8, tag=f"mF8{w}",
                            name=f"mF8{w}") for w in range(2)]

            def mloc(c2):
                """(wave tile, local pair index) for global pair c2."""
                w = (c2 // 4) % 2
                return mF8w[w], (c2 % 4) + 4 * (c2 // 8)
            hnLoc = st.tile([H, r], bf16, tag="hnLoc", name="hnLoc")

            # ---- resident x^T fp8 pair tiles, in consumption order ----
            # (loaded AFTER weights/state so those small DMAs aren't stuck
            #  behind 8MB of x upload; x rides the idle gpsimd DMA queue)
            xtp = [None] * kc2
            order = [p for w in range(2) for p in
                     (list(range(4 * w, 4 * w + 4))
                      + list(range(kc2 // 2 + 4 * w, kc2 // 2 + 4 * w + 4)))]
            for c2 in order:
                t = st.tile([128, 2, r], fp8, tag=f"xtp{c2}", name=f"xtp{c2}")
                for j in range(2):
                    c = 2 * c2 + j
                    nc.gpsimd.dma_start(t[:, j, :],
                                        xt_in[c * 128:(c + 1) * 128, :])
                xtp[c2] = t

            # ---- DRAM bounce buffers for collectives (m, fp8) ----
            cc_in = [dr.tile([128, 2 * ntt * H], fp8, tag=f"cci{w}", bufs=2,
                             name=f"cci{w}") for w in range(2)]
            cc_out = [dr.tile([256, 2 * ntt * H], fp8, tag=f"cco{w}", bufs=2,
                              name=f"cco{w}") for w in range(2)]

            def mlp_pair(r0, r1, it):
                """msg MLP on LOCAL hn blocks r0,r1 -> m8 tiles (interleaved)."""
                m1p, m1s, m2p, m2s, m3p, m8 = {}, {}, {}, {}, {}, {}
                for rb in (r0, r1):
                    src = hnLoc[:, rb * rbsz:(rb + 1) * rbsz]
                    m1p[rb] = ps.tile([H, rbsz], f32, tag="pwork", bufs=2,
                                      name=f"m1p_{it}_{rb}")
                    nc.tensor.matmul(m1p[rb][:], w1gt[:], src,
                                     start=True, stop=True)
                for rb in (r0, r1):
                    m1s[rb] = wk.tile([H, rbsz], bf16, tag="m1s", bufs=2,
                                      name=f"m1s_{it}_{rb}")
                    nc.scalar.activation(m1s[rb][:], m1p[rb][:], AF.Relu,
                                         bias=b1c[:])
                for rb in (r0, r1):
                    m2p[rb] = ps.tile([H, rbsz], f32, tag="pwork", bufs=2,
                                      name=f"m2p_{it}_{rb}")
                    nc.tensor.matmul(m2p[rb][:], w2t[:], m1s[rb][:],
                                     start=True, stop=True)
                for rb in (r0, r1):
                    m2s[rb] = wk.tile([H, rbsz], bf16, tag="m2s", bufs=2,
                                      name=f"m2s_{it}_{rb}")
                    nc.vector.tensor_scalar(m2s[rb][:], m2p[rb][:], b2c[:],
                                            0.0, op0=ALU.add, op1=ALU.max)
                for rb in (r0, r1):
                    m3p[rb] = ps.tile([128, ntt * H], f32, tag="pwork", bufs=2,
                                      name=f"m3p_{it}_{rb}")
                    for t in range(ntt):
                        nc.tensor.matmul(m3p[rb][:, t * H:(t + 1) * H],
                                         m2s[rb][:, t * 128:(t + 1) * 128],
                                         w3t[:], start=True, stop=True)
                for rb in (r0, r1):
                    m8[rb] = wk.tile([128, ntt * H], fp8, tag="m8", bufs=2,
                                     name=f"m8_{it}_{rb}")
                    nc.vector.tensor_scalar(m8[rb][:], m3p[rb][:], S, None,
                                            op0=ALU.mult)
                return m8

            def wave_gather(w, m8, it):
                """AllGather wave w's two m8 blocks; land as mF8 chunk pairs."""
                r0, r1 = 2 * w, 2 * w + 1
                nc.sync.dma_start(cc_in[w][:, 0:ntt * H], m8[r0][:])
                nc.sync.dma_start(cc_in[w][:, ntt * H:2 * ntt * H], m8[r1][:])
                nc.gpsimd.collective_compute(
                    "AllGather", ALU.bypass, replica_groups=GROUPS,
                    ins=[cc_in[w][:].opt()], outs=[cc_out[w][:].opt()])
                # both rank halves land as ready-to-use chunk pairs
                nc.sync.dma_start(mF8w[w][:, 0:4, :, :], cc_out[w][0:128, :])
                nc.sync.dma_start(mF8w[w][:, 4:8, :, :], cc_out[w][128:256, :])

            def chunk_block(orb, c2s, msgp, done, it):
                """Accumulate fp8 DoubleRow chunk pairs into ONE msg psum."""
                for c2 in c2s:
                    done[orb] += 1
                    mt, li = mloc(c2)
                    nc.tensor.matmul(
                        msgp[orb][:],
                        mt[:, li, :, :],
                        xtp[c2][:, :, orb * rbsz:(orb + 1) * rbsz],
                        start=False, stop=(done[orb] == kc2),
                        perf_mode=DR)

            def lstm_a(rb, mp, it, gact):
                """msgb + gate matmuls + activations for block rb."""
                msgb = wk.tile([H, rbsz], bf16, tag="msgb", bufs=2,
                               name=f"msgb_{it}_{rb}")
                nc.vector.tensor_scalar(msgb[:], mp[:], 1.0 / S, None,
                                        op0=ALU.mult)
                gact[rb] = []
                for g in range(4):
                    gp = ps.tile([H, rbsz], f32, tag="pwork", bufs=2,
                                 name=f"gp_{it}_{rb}_{g}")
                    nc.tensor.matmul(gp[:], wiht[:, g * H:(g + 1) * H],
                                     msgb[:], start=True, stop=False)
                    nc.tensor.matmul(gp[:], whht[:, g * H:(g + 1) * H],
                                     h_rb[rb][:], start=False, stop=True)
                    ga = wk.tile([H, rbsz], bf16, tag=f"ga{g}", bufs=2,
                                 name=f"ga_{it}_{rb}_{g}")
                    nc.scalar.activation(
                        ga[:], gp[:],
                        AF.Tanh if g == 2 else AF.Sigmoid,
                        bias=bgc[:, g:g + 1])
                    gact[rb].append(ga)

            def lstm_c(rb, it, gact):
                """c update on DVE for block rb."""
                si, sf, tg, so = gact[rb]
                t1 = wk.tile([H, rbsz], bf16, tag="t1", bufs=2,
                             name=f"t1_{it}_{rb}")
                nc.vector.tensor_tensor(t1[:], sf[:], c_rb[rb][:], ALU.mult)
                t2 = wk.tile([H, rbsz], bf16, tag="t2", bufs=2,
                             name=f"t2_{it}_{rb}")
                nc.vector.tensor_tensor(t2[:], si[:], tg[:], ALU.mult)
                nc.vector.tensor_tensor(c_rb[rb][:], t1[:], t2[:], ALU.add)

            def lstm_h(rb, it, gact):
                """tanh(c) + h update for block rb."""
                so = gact[rb][3]
                tnc = wk.tile([H, rbsz], bf16, tag="tnc", bufs=2,
                              name=f"tnc_{it}_{rb}")
                nc.scalar.activation(tnc[:], c_rb[rb][:], AF.Tanh)
                nc.vector.tensor_tensor(h_rb[rb][:], so[:], tnc[:], ALU.mult)

            def ln_trp(rb, it, trps):
                """transpose 4 h tiles of block rb into one psum bank."""
                tb = ps.tile([128, 2 * ntt, 128], bf16, tag="ptr", bufs=2,
                             name=f"tb_{it}_{rb}")
                trp4 = tb[:, 0:ntt, :]
                for t in range(ntt):
                    nc.tensor.transpose(
                        trp4[:, t, :], h_rb[rb][:, t * 128:(t + 1) * 128],
                        ident[:])
                trps[rb] = tb

            def ln_stats(rb, it, trps, mvs):
                """bn stats + bit-trick rsqrt (DVE; no scalar sqrt)."""
                trp4 = trps[rb][:, 0:ntt, :]
                st6 = wk.tile([128, ntt, 6], f32, tag="st6", bufs=2,
                              name=f"st6_{it}_{rb}")
                for t in range(ntt):
                    nc.vector.bn_stats(st6[:, t, :], trp4[:, t, :])
                mvb = wk.tile([128, ntt, 2], f32, tag="mvb", bufs=2,
                              name=f"mvb_{it}_{rb}")
                for t in range(ntt):
                    nc.vector.bn_aggr(mvb[:, t, :], st6[:, t, :])
                ve = wk.tile([128, ntt], f32, tag="ve", bufs=2,
                             name=f"ve_{it}_{rb}")
                nc.vector.tensor_scalar(ve[:], mvb[:, :, 1], EPS, None,
                                        op0=ALU.add)
                y0 = wk.tile([128, ntt], f32, tag="y0", bufs=2,
                             name=f"y0_{it}_{rb}")
                ti = wk.tile([128, ntt], i32, tag="ti", bufs=2,
                             name=f"ti_{it}_{rb}")
                nc.vector.tensor_scalar(ti[:], ve[:].bitcast(i32), 1, None,
                                        op0=ALU.logical_shift_right)
                nc.vector.tensor_scalar(y0[:].bitcast(i32), ti[:], MAGIC, -1,
                                        op0=ALU.subtract, op1=ALU.mult)
                aa = wk.tile([128, ntt], f32, tag="aa", bufs=2,
                             name=f"aa_{it}_{rb}")
                nc.vector.tensor_tensor(aa[:], y0[:], y0[:], ALU.mult)
                nc.vector.tensor_tensor(aa[:], ve[:], aa[:], ALU.mult)
                nc.vector.tensor_scalar(aa[:], aa[:], -0.5, 1.5,
                                        op0=ALU.mult, op1=ALU.add)
                sc4 = wk.tile([128, ntt], f32, tag="sc4", bufs=2,
                              name=f"sc4_{it}_{rb}")
                nc.vector.tensor_tensor(sc4[:], y0[:], aa[:], ALU.mult)
                mvs[rb] = (mvb, sc4)

            def ln_hnp(rb, it, trps, mvs):
                """normalize + transpose back + hnLoc copy for block rb."""
                trp4 = trps[rb][:, 0:ntt, :]
                mvb, sc4 = mvs[rb]
                hnp4 = trps[rb][:, ntt:2 * ntt, :]
                for t in range(ntt):
                    hnr = wk.tile([128, 128], bf16, tag="hnr", bufs=3,
                                  name=f"hnr_{it}_{rb}_{t}")
                    nc.vector.tensor_scalar(hnr[:], trp4[:, t, :],
                                            mvb[:, t, 0:1], sc4[:, t:t + 1],
                                            op0=ALU.subtract, op1=ALU.mult)
                    nc.tensor.transpose(hnp4[:, t, :], hnr[:], ident[:])
                sl = slice(rb * rbsz, (rb + 1) * rbsz)
                nc.vector.tensor_copy(hnLoc[:, sl], hnp4[:])

            # ================= main loop =================
            for it in range(1, iters + 1):
                # rank-1 bias matmuls open each msg psum accumulation group
                msgp = [ps.tile([H, rbsz], f32, tag="pmsg", bufs=4,
                                name=f"msg_{it}_{orb}") for orb in range(nrb)]
                done = [0] * nrb
                if it == 1:
                    for orb in range(nrb):
                        sl = slice(orb * rbsz, (orb + 1) * rbsz)
                        nc.tensor.matmul(msgp[orb][:], m0_sb[:], rs_sb[:, sl],
                                         start=True, stop=True)
                gact, trps, mvs = {}, {}, {}
                w0p = list(range(0, 4)) + list(range(kc2 // 2, kc2 // 2 + 4))
                w1p = [p + 4 for p in w0p]
                if it > 1:
                    for orb in range(nrb):
                        sl = slice(orb * rbsz, (orb + 1) * rbsz)
                        nc.tensor.matmul(msgp[orb][:], b3r[:], rs_sb[:, sl],
                                         start=True, stop=False)
                    # ORB-MAJOR chunks: wave-0 pairs for every block first,
                    # then per block its wave-1 pairs immediately followed by
                    # that block's gate matmuls -- so LSTM/LN/MLP of early
                    # blocks overlap the remaining chunk matmuls and the
                    # AllGathers fire while the PE is still busy
                    for orb in range(nrb):
                        chunk_block(orb, w0p, msgp, done, it)
                    for orb in range(nrb):
                        chunk_block(orb, w1p, msgp, done, it)
                        lstm_a(orb, msgp[orb], it, gact)
                else:
                    for rb in range(nrb):
                        lstm_a(rb, msgp[rb], it, gact)
                for rb in range(nrb):
                    lstm_c(rb, it, gact)
                for rb in range(nrb):
                    lstm_h(rb, it, gact)
                for rb in range(nrb):
                    ln_trp(rb, it, trps)
                    ln_stats(rb, it, trps, mvs)
                    ln_hnp(rb, it, trps, mvs)
                    if rb % 2 == 1 and it < iters:
                        m8 = mlp_pair(rb - 1, rb, it)
                        wave_gather(rb // 2, m8, it)

            # ================= vote =================
            for rb in range(nrb):
                sl = slice(rb * rbsz, (rb + 1) * rbsz)
                v1p = ps.tile([H, rbsz], f32, tag="pmsg", bufs=4,
                              name=f"v1p_{rb}")
                nc.tensor.matmul(v1p[:], vw1gt[:], hnLoc[:, sl],
                                 start=True, stop=True)
                v1s = wk.tile([H, rbsz], bf16, tag="v1s", bufs=1,
                              name=f"v1s_{rb}")
                nc.scalar.activation(v1s[:], v1p[:], AF.Relu, bias=vb1c[:])
                v2p = ps.tile([H, rbsz], f32, tag="pmsg", bufs=4,
                              name=f"v2p_{rb}")
                nc.tensor.matmul(v2p[:], vw2t[:], v1s[:], start=True, stop=True)
                v2s = wk.tile([H, rbsz], bf16, tag="v2s", bufs=1,
                              name=f"v2s_{rb}")
                nc.scalar.activation(v2s[:], v2p[:], AF.Relu, bias=vb2c[:])
                vop = ps.tile([1, rbsz], f32, tag="pwork", bufs=2,
                              name=f"vop_{rb}")
                nc.tensor.matmul(vop[:], vw3t[:], v2s[:], start=True, stop=True)
                vos = wk.tile([1, rbsz], f32, tag="vos", bufs=1,
                              name=f"vos_{rb}")
                nc.scalar.activation(vos[:], vop[:], AF.Copy)
                nc.sync.dma_start(votes_out[:, sl], vos[:])

    nc.compile()
    return nc


_NC_CACHE = {}


def _get_module():
    key = (N, ITERS)
    if key not in _NC_CACHE:
        _NC_CACHE[key] = build_module(N, ITERS)
    return _NC_CACHE[key]


def _host_prep(inputs):
    """Fold weights, run init MLP, build per-core in_maps."""
    g = lambda s: np.asarray(inputs[s], np.float32)
    x = g("x")
    k, n = g("k"), g("n")

    nk = np.stack([k, n], 1)
    a = np.maximum(nk @ g("init_w1").T + g("init_b1"), 0)
    a = np.maximum(a @ g("init_w2").T + g("init_b2"), 0)
    init0 = a @ g("init_w3").T + g("init_b3")          # [B, H]

    ln_g, ln_b = g("ln_g"), g("ln_b")
    mu0 = init0.mean(1, keepdims=True)
    var0 = init0.var(1, keepdims=True)
    embed0 = (init0 - mu0) / np.sqrt(var0 + EPS) * ln_g + ln_b
    t = np.maximum(embed0 @ g("msg_w1").T + g("msg_b1"), 0)
    t = np.maximum(t @ g("msg_w2").T + g("msg_b2"), 0)
    m0eff = t @ g("msg_w3").T + g("msg_b3")            # [B, H]

    com = {
        "w1gt": (g("msg_w1") * ln_g[None, :]).T.astype(BF),
        "w2t": g("msg_w2").T.astype(BF),
        "w3t": g("msg_w3").T.astype(BF),
        "vw1gt": (g("vote_w1") * ln_g[None, :]).T.astype(BF),
        "vw2t": g("vote_w2").T.astype(BF),
        "vw3t": g("vote_w3").T.astype(BF),              # [H, 1]
        "wiht": g("lstm_wih").T.astype(BF),
        "whht": g("lstm_whh").T.astype(BF),
        "b1c": (g("msg_w1") @ ln_b + g("msg_b1")).reshape(H, 1).astype(np.float32),
        "b2c": g("msg_b2").reshape(H, 1).astype(np.float32),
        "vb1c": (g("vote_w1") @ ln_b + g("vote_b1")).reshape(H, 1).astype(np.float32),
        "vb2c": g("vote_b2").reshape(H, 1).astype(np.float32),
        "bgc": (g("lstm_bih") + g("lstm_bhh")).reshape(4, H).T.astype(np.float32).copy(),
        "b3r": (g("msg_b3") * S).reshape(1, H).astype(BF),
        "ident": np.eye(H, dtype=BF),
    }

    in_maps = []
    for core in range(NCORES):
        b = core // 2
        r0 = (core % 2) * R
        xs = x[b][r0:r0 + R, :]                         # [R, N] local rows
        x8 = np.clip(xs.T, -240.0, 240.0).astype(F8)    # [N, R] fp8 (TRN e4)
        m = dict(com)
        m["xt8"] = np.ascontiguousarray(x8)
        m["rs"] = x8.astype(np.float32).sum(0).reshape(1, R).astype(BF)
        m["h0"] = np.ascontiguousarray(
            np.broadcast_to(init0[b][:, None], (H, R))).astype(BF)
        m["m0"] = (m0eff[b] * S).reshape(1, H).astype(BF)
        in_maps.append(m)
    return in_maps


def kernel(**inputs):
    nc = _get_module()
    in_maps = _host_prep(inputs)
    res = run_bass_kernel_spmd(nc, in_maps, core_ids=list(range(NCORES)))
    mask = np.asarray(inputs["mask"], np.float64)
    vb3 = float(np.asarray(inputs["vote_b3"], np.float64).reshape(-1)[0])
    out = np.zeros(B, np.float32)
    for b in range(B):
        votes = np.concatenate([
            res.results[2 * b]["votes"].reshape(-1),
            res.results[2 * b + 1]["votes"].reshape(-1),
        ]).astype(np.float64) + vb3
        s = float((votes * mask[b]).sum())
        out[b] = 1.0 / (1.0 + np.exp(-s))
    return out
